# revision 5
# baseline (speedup 1.0000x reference)
"""Trainium2 Bass kernel for nn_DIVLoss (retrieval_knn).

Math: the reference's pred_nn = mean(pred_nn_mat @ nn_label_matrix, axis=1)
collapses exactly (each row of nn_label_matrix holds exactly 10 ones), so
    pred_nn[i] = (10/B) * fsum . qhat[target[i]],   fsum = sum_b fhat[b]
    pred_sel[i] = fhat[perm[i]] . qhat[target[perm[i]]],  perm = stable argsort
    loss = mean_i softplus(SCALE * (pred_nn[i] - pred_sel[i]))

Split: the device does the O(B*D) dot products; the host does data routing
(gathers/permutation/transposes), the norms, fsum, and the final
softplus+mean over 4096 scalars.

Per core (512 perm-sorted rows):
  - VectorE: row-tiles 0,1 of the sel path as direct fp8 STT dots.
  - ScalarE: row-tiles 2,3 via the sum-of-squares identity on bf16.
  - TensorE: the nn path. Because rows are sorted by class, a core's 512
    rows touch only ~125 consecutive classes; we ship qhat for a 160-class
    window (160KB) instead of a per-row gather (512KB) and compute
    v[c] = fsum . qhat[c] as 8 accumulated [128,1]x[128,160] matmuls.
    The host scatters each core's v window into v_full[1000] and gathers
    z_nn[i] = v_full[target[i]].
Queue split (one queue per issuing engine): qSync: xy0,xy1 + outputs,
qScalar: a2,a3, qGpSimd(SWDGE): the W window. All fp8 except the ScalarE
tiles (bf16). Power-of-two scales divide out exactly on the host.
"""

import numpy as np

N_CORES = 8
B = 4096
D = 1024
C = 1000
ROWS = B // N_CORES          # 512 rows per core
T = ROWS // 128              # 4 row-tiles of 128 partitions
CH = D // 128                # 8 contraction chunks for the TensorE path
CW = 160                     # class-window width per core (max span ~134)
SCALE = 100.0
TOPK = 10.0
UN = SCALE * TOPK / B        # nn-path constant folded into fsum

_cache = {}


def _build():
    import concourse.bacc as bacc
    import concourse.mybir as mybir
    import concourse.tile as tile

    f32 = mybir.dt.float32
    bf16 = mybir.dt.bfloat16
    f8 = mybir.dt.float8e4
    AF = mybir.ActivationFunctionType
    ALU = mybir.AluOpType

    nc = bacc.Bacc(
        "TRN2",
        target_bir_lowering=False,
        debug=False,
        enable_asserts=False,
        num_devices=N_CORES,
    )

    PAD = 64       # fsum lives in [0:8); W chunks start 64B-aligned at PAD
    WCOLS = PAD + CH * CW
    wv_d = nc.dram_tensor("wv", [128, WCOLS], f8, kind="ExternalInput")
    xy0_d = nc.dram_tensor("xy0", [128, 2 * D], f8, kind="ExternalInput")
    xy1_d = nc.dram_tensor("xy1", [128, 2 * D], f8, kind="ExternalInput")
    a2_d = nc.dram_tensor("a2", [128, D], bf16, kind="ExternalInput")
    a3_d = nc.dram_tensor("a3", [128, D], bf16, kind="ExternalInput")
    du_d = nc.dram_tensor("du", [128, T], f32, kind="ExternalOutput")
    uo_d = nc.dram_tensor("uo", [1, CW], f32, kind="ExternalOutput")

    with tile.TileContext(nc) as tc:
        with tc.tile_pool(name="sbuf", bufs=1) as pool, tc.tile_pool(
            name="ps", space="PSUM", bufs=1
        ) as pp:
            wv = pool.tile([128, WCOLS], f8, tag="wv")
            xy0 = pool.tile([128, 2 * D], f8, tag="xy0")
            xy1 = pool.tile([128, 2 * D], f8, tag="xy1")
            a2 = pool.tile([128, D], bf16, tag="a2")
            a3 = pool.tile([128, D], bf16, tag="a3")
            du = pool.tile([128, T], f32, tag="du")
            usb = pool.tile([1, CW], f32, tag="usb")
            prod = pool.tile([128, D], bf16, tag="prod")
            sqa = pool.tile([128, D], bf16, tag="sqa")
            pu = pp.tile([1, CW], f32, name="pu", tag="pu")

            # Two HWDGE queues only (SWDGE first-byte latency is ~2.6us).
            # wv first on sync: PE's matmuls are upstream of the uo output;
            # a2 first on scalar so ScalarE (longest chain) starts earliest.
            nc.sync.dma_start(wv[:], wv_d[:])
            nc.sync.dma_start(xy0[:], xy0_d[:])
            nc.sync.dma_start(xy1[:], xy1_d[:])
            nc.scalar.dma_start(a2[:], a2_d[:])
            nc.scalar.dma_start(a3[:], a3_d[:])

            # TensorE: v[c] = sum_ch fsum_ch . W_ch[:, c], accumulated.
            for c in range(CH):
                nc.tensor.matmul(
                    pu[:],
                    wv[:, c : c + 1],
                    wv[:, PAD + c * CW : PAD + (c + 1) * CW],
                    start=(c == 0),
                    stop=(c == CH - 1),
                )

            # VectorE: direct fp8 row dots (tiles 0,1)
            nc.vector.scalar_tensor_tensor(
                prod[:], xy0[:, 0:D], 1.0, xy0[:, D : 2 * D],
                ALU.mult, ALU.mult, accum_out=du[:, 0:1],
            )
            nc.vector.scalar_tensor_tensor(
                prod[:], xy1[:, 0:D], 1.0, xy1[:, D : 2 * D],
                ALU.mult, ALU.mult, accum_out=du[:, 1:2],
            )

            # ScalarE: sum-of-squares row dots (tiles 2,3), then PSUM->SBUF
            nc.scalar.activation(sqa[:], a2[:], AF.Square, accum_out=du[:, 2:3])
            nc.scalar.activation(sqa[:], a3[:], AF.Square, accum_out=du[:, 3:4])
            nc.scalar.copy(usb[:], pu[:])

            nc.scalar.dma_start(uo_d[:], usb[:])
            nc.sync.dma_start(du_d[:], du[:])

    nc.compile()
    return nc


def _host_prep(feature, query, target):
    import ml_dtypes

    f8 = ml_dtypes.float8_e4m3
    bf = ml_dtypes.bfloat16

    f = feature.astype(np.float64)
    q = query.astype(np.float64)
    t = np.asarray(target).astype(np.int64)
    perm = np.argsort(t, kind="stable")
    ts = t[perm]

    nf = np.sqrt((f * f).sum(1))
    nq = np.sqrt((q * q).sum(1))
    qhat = q / nq[:, None]
    fsum = (f / nf[:, None]).sum(0)

    c2 = SCALE / (nf[perm] * nq[t[perm]])
    x = f[perm] * (8.0 * c2)[:, None]   # sel-path lhs, scale folded (2^3)
    y = q[t[perm]]                      # sel-path rhs, raw
    x8 = np.ascontiguousarray(x.astype(f8))
    y8 = np.ascontiguousarray(y.astype(f8))
    a16 = np.ascontiguousarray((x + y).astype(bf))
    h = (x * x).sum(1) + (y * y).sum(1)  # exact, host-removed

    qh8 = (qhat * 32.0).astype(f8)                           # 2^5 folded
    fsb8 = (fsum * UN).astype(f8)
    fsw = np.zeros((128, 64), dtype=f8)                      # 64B-aligned pad
    fsw[:, 0:CH] = fsb8.reshape(CH, 128).T

    # per-core class windows (rows are perm-sorted so classes are contiguous)
    bases = []
    for k in range(N_CORES):
        seg = ts[k * ROWS : (k + 1) * ROWS]
        lo, hi = int(seg[0]), int(seg[-1])
        assert hi - lo + 1 <= CW, (lo, hi)
        base = min(lo, C - CW)
        bases.append(base)
    return x8, y8, a16, h, qh8, fsw, bases, t


def kernel(feature, query, target):
    feature = np.ascontiguousarray(np.asarray(feature), dtype=np.float32)
    query = np.ascontiguousarray(np.asarray(query), dtype=np.float32)
    target = np.asarray(target)

    if "nc" not in _cache:
        _cache["nc"] = _build()
    nc = _cache["nc"]

    x8, y8, a16, h, qh8, fsw, bases, t = _host_prep(feature, query, target)

    in_maps = []
    for k in range(N_CORES):
        s0 = k * ROWS
        r = [slice(s0 + tt * 128, s0 + (tt + 1) * 128) for tt in range(T)]
        # W chunks: [128 (d within chunk), CH*CW], chunk-major columns
        wk = qh8[bases[k] : bases[k] + CW]            # [CW, 1024]
        # wk.T is [1024, CW]; chunk c is rows [128c:128c+128]
        chunks = np.ascontiguousarray(wk.T).reshape(CH, 128, CW)
        wvrow = np.concatenate(
            [fsw.view(np.uint8)]
            + [np.ascontiguousarray(chunks[c]).view(np.uint8) for c in range(CH)],
            axis=1,
        )
        in_maps.append(
            {
                "wv": np.ascontiguousarray(wvrow).view(qh8.dtype),
                "xy0": np.ascontiguousarray(
                    np.concatenate([x8[r[0]].view(np.uint8), y8[r[0]].view(np.uint8)], axis=1)
                ).view(x8.dtype),
                "xy1": np.ascontiguousarray(
                    np.concatenate([x8[r[1]].view(np.uint8), y8[r[1]].view(np.uint8)], axis=1)
                ).view(x8.dtype),
                "a2": np.ascontiguousarray(a16[r[2]]),
                "a3": np.ascontiguousarray(a16[r[3]]),
            }
        )

    from concourse.bass_utils import run_bass_kernel_spmd

    res = run_bass_kernel_spmd(
        nc,
        in_maps,
        core_ids=list(range(N_CORES)),
        trace=bool(getattr(kernel, "_trace", False)),
        tmpdir=getattr(kernel, "_tmpdir", None),
    )
    kernel.last_results = res

    z_sel = np.empty(B)
    v_full = np.zeros(C)
    for k in range(N_CORES):
        s0 = k * ROWS
        du = res.results[k]["du"].astype(np.float64)   # [128, T]
        uo = res.results[k]["uo"].astype(np.float64)   # [1, CW]
        v_full[bases[k] : bases[k] + CW] = uo[0] / 32.0
        for tt in range(T):
            rows = slice(s0 + tt * 128, s0 + (tt + 1) * 128)
            if tt < 2:
                z_sel[rows] = du[:, tt] / 8.0
            else:
                z_sel[rows] = (du[:, tt] - h[rows]) / 16.0

    z_nn = v_full[t]
    loss = np.mean(np.logaddexp(0.0, z_nn - z_sel))
    return np.asarray(loss, dtype=np.float32)


# revision 8
# speedup vs baseline: 1.0300x; 1.0300x over previous
"""Trainium2 Bass kernel for nn_DIVLoss (retrieval_knn).

Math: the reference's pred_nn = mean(pred_nn_mat @ nn_label_matrix, axis=1)
collapses exactly (each row of nn_label_matrix holds exactly 10 ones), so
    pred_nn[i] = (10/B) * fsum . qhat[target[i]],   fsum = sum_b fhat[b]
    pred_sel[i] = fhat[perm[i]] . qhat[target[perm[i]]],  perm = stable argsort
    loss = mean_i softplus(SCALE * (pred_nn[i] - pred_sel[i]))

Device/host split: the device does the O(B*D) dot products; the host does
routing (argsort/gathers/transposes), norms, fsum, and the final
softplus+mean over 4096 scalars.

Key structure: rows are shipped perm-sorted, so a core's 512 rows span only
~125 consecutive classes (window CW=160) and each 128-row tile fits a
96-class window on a fixed cross-core grid. Both the sel and nn paths then
share ONE small qhat-window matrix W [1024, CW] per core:
  - TensorE computes P_t = xs_t @ W[:, g_t:g_t+96] (4 tiles, fp8,
    8 accumulated chunk matmuls each, stationary = transposed x chunks)
    and v = (UN*fsum) @ W as a separate accumulation.
  - VectorE extracts z_sel per row as an STT with a one-hot fp8 mask:
    du[:,t] = sum_c P_t[i,c]*M_t[i,c] = P_t[i, rel(i)]  (exact).
  - ScalarE only copies v PSUM->SBUF (no activations -> no act table).
  - Dummy matmuls at body start keep the PE busy through the DMA wait so
    the HAM clock gate releases (1.2 -> 2.4 GHz) before the real matmuls.
Per-core input is ~768KB (vs 1.58MB for the per-row form): wv 208KB +
4 x 128KB transposed x tiles + 48KB masks. Scales are powers of two and
divide out exactly on the host (z = P/32).
"""

import numpy as np

N_CORES = 8
B = 4096
D = 1024
C = 1000
ROWS = B // N_CORES          # 512 rows per core
T = ROWS // 128              # 4 row-tiles of 128 partitions
CH = D // 128                # 8 contraction chunks
CW = 160                     # per-core class window (max span ~134)
TW = 96                      # per-tile class window on the shared grid
SCALE = 100.0
TOPK = 10.0
UN = SCALE * TOPK / B        # nn-path constant folded into fsum
PAD = 64                     # fsum in wv[:, 0:8); W chunks 64B-aligned

_cache = {}


def _build(grid):
    """grid: T compile-time tile-window offsets shared by all cores."""
    import concourse.bacc as bacc
    import concourse.mybir as mybir
    import concourse.tile as tile

    f32 = mybir.dt.float32
    f8 = mybir.dt.float8e4
    bf16 = mybir.dt.bfloat16
    ALU = mybir.AluOpType

    nc = bacc.Bacc(
        "TRN2",
        target_bir_lowering=False,
        debug=False,
        enable_asserts=False,
        num_devices=N_CORES,
    )

    WCOLS = PAD + CH * CW
    wv_d = nc.dram_tensor("wv", [128, WCOLS], f8, kind="ExternalInput")
    xt01_d = nc.dram_tensor("xt01", [128, 2 * CH * 128], f8, kind="ExternalInput")
    xt23_d = nc.dram_tensor("xt23", [128, 2 * CH * 128], f8, kind="ExternalInput")
    mk_d = nc.dram_tensor("mk", [128, T * TW], f8, kind="ExternalInput")
    du_d = nc.dram_tensor("du", [128, T], f32, kind="ExternalOutput")
    uo_d = nc.dram_tensor("uo", [1, CW], f32, kind="ExternalOutput")

    with tile.TileContext(nc) as tc:
        with tc.tile_pool(name="sbuf", bufs=1) as pool, tc.tile_pool(
            name="ps", space="PSUM", bufs=1
        ) as pp:
            wv = pool.tile([128, WCOLS], f8, tag="wv")
            xt01 = pool.tile([128, 2 * CH * 128], f8, tag="xt01")
            xt23 = pool.tile([128, 2 * CH * 128], f8, tag="xt23")
            mk = pool.tile([128, T * TW], f8, tag="mk")
            dmy = pool.tile([128, 512], bf16, tag="dmy")
            du = pool.tile([128, T], f32, tag="du")
            usb = pool.tile([1, CW], f32, tag="usb")
            prodx = pool.tile([128, TW], f32, tag="prodx")
            pu = pp.tile([1, CW], f32, name="pu", tag="pu")
            pdm = pp.tile([1, 512], f32, name="pdm", tag="pdm")
            pt = [
                pp.tile([128, 512], f32, name=f"pt{t}", tag=f"pt{t}")
                for t in range(T)
            ]

            # inputs: sync gets the x tiles, scalar gets W + masks
            nc.scalar.dma_start(wv[:], wv_d[:])
            nc.sync.dma_start(xt01[:], xt01_d[:])
            nc.sync.dma_start(xt23[:], xt23_d[:])
            nc.scalar.dma_start(mk[:], mk_d[:])

            # PE warm-up: keep the array busy from body start so the HAM
            # clock gate releases before the real matmuls arrive.
            nc.vector.memset(dmy[:], 0.0)
            for i in range(7):
                nc.tensor.matmul(pdm[:], dmy[:, i : i + 1], dmy[:],
                                 start=True, stop=True)

            # nn path: v = fsw . W  (PSUM [1, CW])
            for c in range(CH):
                nc.tensor.matmul(
                    pu[:],
                    wv[:, c : c + 1],
                    wv[:, PAD + c * CW : PAD + (c + 1) * CW],
                    start=(c == 0),
                    stop=(c == CH - 1),
                )

            # sel path: P_t = xs_t @ W[:, g_t : g_t+TW]
            xts = {0: xt01, 1: xt01, 2: xt23, 3: xt23}
            xoff = {0: 0, 1: CH * 128, 2: 0, 3: CH * 128}
            for t in range(T):
                src = xts[t]
                for c in range(CH):
                    nc.tensor.matmul(
                        pt[t][:, 0:TW],
                        src[:, xoff[t] + c * 128 : xoff[t] + (c + 1) * 128],
                        wv[:, PAD + c * CW + grid[t] : PAD + c * CW + grid[t] + TW],
                        start=(c == 0),
                        stop=(c == CH - 1),
                    )

            # VectorE: exact one-hot extraction -> du[:, t]
            for t in range(T):
                nc.vector.scalar_tensor_tensor(
                    prodx[:], pt[t][:, 0:TW], 1.0, mk[:, t * TW : (t + 1) * TW],
                    ALU.mult, ALU.mult, accum_out=du[:, t : t + 1],
                )

            # ScalarE: v PSUM -> SBUF, then out
            nc.scalar.copy(usb[:], pu[:])
            nc.scalar.dma_start(uo_d[:], usb[:])
            nc.sync.dma_start(du_d[:], du[:])

    nc.compile()
    return nc


def _host_prep(feature, query, target):
    import ml_dtypes

    f8 = ml_dtypes.float8_e4m3

    f = feature.astype(np.float64)
    q = query.astype(np.float64)
    t = np.asarray(target).astype(np.int64)
    perm = np.argsort(t, kind="stable")
    ts = t[perm]

    nf = np.sqrt((f * f).sum(1))
    nq = np.sqrt((q * q).sum(1))
    qhat = q / nq[:, None]
    fsum = (f / nf[:, None]).sum(0)

    xs = (f[perm] / nf[perm, None]) * SCALE      # SCALE * fhat, perm order
    x8 = np.ascontiguousarray(xs.astype(f8))

    qh8 = (qhat * 32.0).astype(f8)               # 2^5 folded, shared W
    fsb8 = (fsum * UN).astype(f8)
    fsw = np.zeros((128, PAD), dtype=f8)
    fsw[:, 0:CH] = fsb8.reshape(CH, 128).T

    # per-core window bases + the shared per-tile grid
    bases = []
    lo_kt = np.zeros((N_CORES, T), dtype=np.int64)
    hi_kt = np.zeros((N_CORES, T), dtype=np.int64)
    for k in range(N_CORES):
        seg = ts[k * ROWS : (k + 1) * ROWS]
        lo, hi = int(seg[0]), int(seg[-1])
        assert hi - lo + 1 <= CW, (lo, hi)
        base = min(lo, C - CW)
        bases.append(base)
        for tt in range(T):
            tseg = seg[tt * 128 : (tt + 1) * 128]
            lo_kt[k, tt] = int(tseg[0]) - base
            hi_kt[k, tt] = int(tseg[-1]) - base
    grid = []
    for tt in range(T):
        gmin = max(0, int(hi_kt[:, tt].max()) - TW + 1)
        gmax = min(CW - TW, int(lo_kt[:, tt].min()))
        assert gmin <= gmax, (tt, gmin, gmax)
        grid.append((gmin + gmax) // 2)

    rel = np.empty(B, dtype=np.int64)
    for k in range(N_CORES):
        for tt in range(T):
            rows = slice(k * ROWS + tt * 128, k * ROWS + (tt + 1) * 128)
            rel[rows] = ts[rows] - bases[k] - grid[tt]
    assert rel.min() >= 0 and rel.max() < TW
    return x8, qh8, fsw, bases, tuple(grid), rel, t


def kernel(feature, query, target):
    feature = np.ascontiguousarray(np.asarray(feature), dtype=np.float32)
    query = np.ascontiguousarray(np.asarray(query), dtype=np.float32)
    target = np.asarray(target)

    x8, qh8, fsw, bases, grid, rel, t = _host_prep(feature, query, target)

    if grid not in _cache:
        _cache[grid] = _build(grid)
    nc = _cache[grid]

    import ml_dtypes
    f8d = np.dtype(ml_dtypes.float8_e4m3)

    ridx = np.arange(128)
    in_maps = []
    for k in range(N_CORES):
        s0 = k * ROWS
        wk = qh8[bases[k] : bases[k] + CW]            # [CW, 1024]
        chunks = np.ascontiguousarray(wk.T).reshape(CH, 128, CW)
        wvrow = np.concatenate(
            [fsw.view(np.uint8)]
            + [np.ascontiguousarray(chunks[c]).view(np.uint8) for c in range(CH)],
            axis=1,
        )

        xtp = []
        mks = []
        for tt in range(T):
            rows = slice(s0 + tt * 128, s0 + (tt + 1) * 128)
            xtT = np.ascontiguousarray(x8[rows].view(np.uint8).T)  # [1024,128]
            xtp.append(xtT.reshape(CH, 128, 128))
            m = np.zeros((128, TW), dtype=f8d)
            m[ridx, rel[rows]] = 1.0
            mks.append(m.view(np.uint8))
        xt01 = np.ascontiguousarray(
            np.concatenate([xtp[0], xtp[1]], axis=0)
            .transpose(1, 0, 2)
            .reshape(128, 2 * CH * 128)
        )
        xt23 = np.ascontiguousarray(
            np.concatenate([xtp[2], xtp[3]], axis=0)
            .transpose(1, 0, 2)
            .reshape(128, 2 * CH * 128)
        )
        mkc = np.ascontiguousarray(np.concatenate(mks, axis=1))
        in_maps.append(
            {
                "wv": np.ascontiguousarray(wvrow).view(f8d),
                "xt01": xt01.view(f8d),
                "xt23": xt23.view(f8d),
                "mk": mkc.view(f8d),
            }
        )

    from concourse.bass_utils import run_bass_kernel_spmd

    res = run_bass_kernel_spmd(
        nc,
        in_maps,
        core_ids=list(range(N_CORES)),
        trace=bool(getattr(kernel, "_trace", False)),
        tmpdir=getattr(kernel, "_tmpdir", None),
    )
    kernel.last_results = res

    z_sel = np.empty(B)
    v_full = np.zeros(C)
    for k in range(N_CORES):
        s0 = k * ROWS
        du = res.results[k]["du"].astype(np.float64)   # [128, T]
        uo = res.results[k]["uo"].astype(np.float64)   # [1, CW]
        v_full[bases[k] : bases[k] + CW] = uo[0] / 32.0
        for tt in range(T):
            rows = slice(s0 + tt * 128, s0 + (tt + 1) * 128)
            z_sel[rows] = du[:, tt] / 32.0

    # z_sel is in perm-row order; z_nn in original order — matching the
    # reference's own (faithfully replicated) row pairing.
    z_nn = v_full[t]
    loss = np.mean(np.logaddexp(0.0, z_nn - z_sel))
    return np.asarray(loss, dtype=np.float32)


# revision 9
# speedup vs baseline: 1.1351x; 1.1020x over previous
"""Trainium2 Bass kernel for nn_DIVLoss (retrieval_knn).

Math: the reference's pred_nn = mean(pred_nn_mat @ nn_label_matrix, axis=1)
collapses exactly (each row of nn_label_matrix holds exactly 10 ones), so
    pred_nn[i] = (10/B) * fsum . qhat[target[i]],   fsum = sum_b fhat[b]
    pred_sel[i] = fhat[perm[i]] . qhat[target[perm[i]]],  perm = stable argsort
    loss = mean_i softplus(SCALE * (pred_nn[i] - pred_sel[i]))

Device/host split: the device does the O(B*D) dot products; the host does
routing (argsort/gathers/transposes), norms, fsum, and the final
softplus+mean over 4096 scalars.

Key structure: rows are shipped perm-sorted, so a core's 512 rows span only
~125 consecutive classes (window CW=160) and each 128-row tile fits a
96-class window on a fixed cross-core grid. Both the sel and nn paths then
share ONE small qhat-window matrix W [1024, CW] per core:
  - TensorE computes P_t = xs_t @ W[:, g_t:g_t+96] (4 tiles, fp8,
    8 accumulated chunk matmuls each, stationary = transposed x chunks)
    and v = (UN*fsum) @ W as a separate accumulation.
  - VectorE extracts z_sel per row as an STT with a one-hot fp8 mask:
    du[:,t] = sum_c P_t[i,c]*M_t[i,c] = P_t[i, rel(i)]  (exact).
  - ScalarE only copies v PSUM->SBUF (no activations -> no act table).
  - Dummy matmuls at body start keep the PE busy through the DMA wait so
    the HAM clock gate releases (1.2 -> 2.4 GHz) before the real matmuls.
Per-core input is ~768KB (vs 1.58MB for the per-row form): wv 208KB +
4 x 128KB transposed x tiles + 48KB masks. Scales are powers of two and
divide out exactly on the host (z = P/32).
"""

import numpy as np

N_CORES = 8
B = 4096
D = 1024
C = 1000
ROWS = B // N_CORES          # 512 rows per core
T = ROWS // 128              # 4 row-tiles of 128 partitions
CH = D // 128                # 8 contraction chunks
CW = 160                     # per-core class window (max span ~134)
TW = 96                      # per-tile class window on the shared grid
SCALE = 100.0
TOPK = 10.0
UN = SCALE * TOPK / B        # nn-path constant folded into fsum
PAD = 64                     # fsum in wv[:, 0:8); W chunks 64B-aligned

_cache = {}


def _build(grid):
    """grid: T compile-time tile-window offsets shared by all cores."""
    import concourse.bacc as bacc
    import concourse.mybir as mybir
    import concourse.tile as tile

    f32 = mybir.dt.float32
    f8 = mybir.dt.float8e4
    bf16 = mybir.dt.bfloat16
    ALU = mybir.AluOpType

    nc = bacc.Bacc(
        "TRN2",
        target_bir_lowering=False,
        debug=False,
        enable_asserts=False,
        num_devices=N_CORES,
    )

    WCOLS = PAD + CH * CW
    wv_d = nc.dram_tensor("wv", [128, WCOLS], f8, kind="ExternalInput")
    xt01_d = nc.dram_tensor("xt01", [128, 2 * CH * 128], f8, kind="ExternalInput")
    xt23_d = nc.dram_tensor("xt23", [128, 2 * CH * 128], f8, kind="ExternalInput")
    mk_d = nc.dram_tensor("mk", [128, T * TW], f8, kind="ExternalInput")
    du_d = nc.dram_tensor("du", [128, T], f32, kind="ExternalOutput")
    uo_d = nc.dram_tensor("uo", [1, CW], f32, kind="ExternalOutput")

    with tile.TileContext(nc) as tc:
        with tc.tile_pool(name="sbuf", bufs=1) as pool, tc.tile_pool(
            name="ps", space="PSUM", bufs=1
        ) as pp:
            wv = pool.tile([128, WCOLS], f8, tag="wv")
            xt01 = pool.tile([128, 2 * CH * 128], f8, tag="xt01")
            xt23 = pool.tile([128, 2 * CH * 128], f8, tag="xt23")
            mk = pool.tile([128, T * TW], f8, tag="mk")
            dmy = pool.tile([128, 512], bf16, tag="dmy")
            du = pool.tile([128, T], f32, tag="du")
            usb = pool.tile([1, CW], f32, tag="usb")
            prodx = pool.tile([128, TW], f32, tag="prodx")
            pu = pp.tile([1, CW], f32, name="pu", tag="pu")
            pdm = pp.tile([1, 512], f32, name="pdm", tag="pdm")
            pt = [
                pp.tile([128, 512], f32, name=f"pt{t}", tag=f"pt{t}")
                for t in range(T)
            ]

            # inputs: everything PE needs rides the (faster-starting) sync
            # ring in consumption order; masks ride the scalar ring.
            nc.sync.dma_start(wv[:], wv_d[:])
            nc.sync.dma_start(xt01[:], xt01_d[:])
            nc.sync.dma_start(xt23[:], xt23_d[:])
            nc.scalar.dma_start(mk[:], mk_d[:])

            # PE warm-up: keep the array busy from body start so the HAM
            # clock gate releases before the real matmuls arrive.
            nc.vector.memset(dmy[:], 0.0)
            for i in range(5):
                nc.tensor.matmul(pdm[:], dmy[:, i : i + 1], dmy[:],
                                 start=True, stop=True)

            # nn path: v = fsw . W  (PSUM [1, CW])
            for c in range(CH):
                nc.tensor.matmul(
                    pu[:],
                    wv[:, c : c + 1],
                    wv[:, PAD + c * CW : PAD + (c + 1) * CW],
                    start=(c == 0),
                    stop=(c == CH - 1),
                )

            # sel path: P_t = xs_t @ W[:, g_t : g_t+TW]
            xts = {0: xt01, 1: xt01, 2: xt23, 3: xt23}
            xoff = {0: 0, 1: CH * 128, 2: 0, 3: CH * 128}
            for t in range(T):
                src = xts[t]
                for c in range(CH):
                    nc.tensor.matmul(
                        pt[t][:, 0:TW],
                        src[:, xoff[t] + c * 128 : xoff[t] + (c + 1) * 128],
                        wv[:, PAD + c * CW + grid[t] : PAD + c * CW + grid[t] + TW],
                        start=(c == 0),
                        stop=(c == CH - 1),
                    )

            # VectorE: exact one-hot extraction -> du[:, t]
            for t in range(T):
                nc.vector.scalar_tensor_tensor(
                    prodx[:], pt[t][:, 0:TW], 1.0, mk[:, t * TW : (t + 1) * TW],
                    ALU.mult, ALU.mult, accum_out=du[:, t : t + 1],
                )

            # ScalarE: v PSUM -> SBUF, then out
            nc.scalar.copy(usb[:], pu[:])
            nc.scalar.dma_start(uo_d[:], usb[:])
            nc.sync.dma_start(du_d[:], du[:])

    nc.compile()
    return nc


def _host_prep(feature, query, target):
    import ml_dtypes

    f8 = ml_dtypes.float8_e4m3

    f = feature.astype(np.float64)
    q = query.astype(np.float64)
    t = np.asarray(target).astype(np.int64)
    perm = np.argsort(t, kind="stable")
    ts = t[perm]

    nf = np.sqrt((f * f).sum(1))
    nq = np.sqrt((q * q).sum(1))
    qhat = q / nq[:, None]
    fsum = (f / nf[:, None]).sum(0)

    xs = (f[perm] / nf[perm, None]) * SCALE      # SCALE * fhat, perm order
    x8 = np.ascontiguousarray(xs.astype(f8))

    qh8 = (qhat * 32.0).astype(f8)               # 2^5 folded, shared W
    fsb8 = (fsum * UN).astype(f8)
    fsw = np.zeros((128, PAD), dtype=f8)
    fsw[:, 0:CH] = fsb8.reshape(CH, 128).T

    # per-core window bases + the shared per-tile grid
    bases = []
    lo_kt = np.zeros((N_CORES, T), dtype=np.int64)
    hi_kt = np.zeros((N_CORES, T), dtype=np.int64)
    for k in range(N_CORES):
        seg = ts[k * ROWS : (k + 1) * ROWS]
        lo, hi = int(seg[0]), int(seg[-1])
        assert hi - lo + 1 <= CW, (lo, hi)
        base = min(lo, C - CW)
        bases.append(base)
        for tt in range(T):
            tseg = seg[tt * 128 : (tt + 1) * 128]
            lo_kt[k, tt] = int(tseg[0]) - base
            hi_kt[k, tt] = int(tseg[-1]) - base
    grid = []
    for tt in range(T):
        gmin = max(0, int(hi_kt[:, tt].max()) - TW + 1)
        gmax = min(CW - TW, int(lo_kt[:, tt].min()))
        assert gmin <= gmax, (tt, gmin, gmax)
        grid.append((gmin + gmax) // 2)

    rel = np.empty(B, dtype=np.int64)
    for k in range(N_CORES):
        for tt in range(T):
            rows = slice(k * ROWS + tt * 128, k * ROWS + (tt + 1) * 128)
            rel[rows] = ts[rows] - bases[k] - grid[tt]
    assert rel.min() >= 0 and rel.max() < TW
    return x8, qh8, fsw, bases, tuple(grid), rel, t


def kernel(feature, query, target):
    feature = np.ascontiguousarray(np.asarray(feature), dtype=np.float32)
    query = np.ascontiguousarray(np.asarray(query), dtype=np.float32)
    target = np.asarray(target)

    x8, qh8, fsw, bases, grid, rel, t = _host_prep(feature, query, target)

    if grid not in _cache:
        _cache[grid] = _build(grid)
    nc = _cache[grid]

    import ml_dtypes
    f8d = np.dtype(ml_dtypes.float8_e4m3)

    ridx = np.arange(128)
    in_maps = []
    for k in range(N_CORES):
        s0 = k * ROWS
        wk = qh8[bases[k] : bases[k] + CW]            # [CW, 1024]
        chunks = np.ascontiguousarray(wk.T).reshape(CH, 128, CW)
        wvrow = np.concatenate(
            [fsw.view(np.uint8)]
            + [np.ascontiguousarray(chunks[c]).view(np.uint8) for c in range(CH)],
            axis=1,
        )

        xtp = []
        mks = []
        for tt in range(T):
            rows = slice(s0 + tt * 128, s0 + (tt + 1) * 128)
            xtT = np.ascontiguousarray(x8[rows].view(np.uint8).T)  # [1024,128]
            xtp.append(xtT.reshape(CH, 128, 128))
            m = np.zeros((128, TW), dtype=f8d)
            m[ridx, rel[rows]] = 1.0
            mks.append(m.view(np.uint8))
        xt01 = np.ascontiguousarray(
            np.concatenate([xtp[0], xtp[1]], axis=0)
            .transpose(1, 0, 2)
            .reshape(128, 2 * CH * 128)
        )
        xt23 = np.ascontiguousarray(
            np.concatenate([xtp[2], xtp[3]], axis=0)
            .transpose(1, 0, 2)
            .reshape(128, 2 * CH * 128)
        )
        mkc = np.ascontiguousarray(np.concatenate(mks, axis=1))
        in_maps.append(
            {
                "wv": np.ascontiguousarray(wvrow).view(f8d),
                "xt01": xt01.view(f8d),
                "xt23": xt23.view(f8d),
                "mk": mkc.view(f8d),
            }
        )

    from concourse.bass_utils import run_bass_kernel_spmd

    res = run_bass_kernel_spmd(
        nc,
        in_maps,
        core_ids=list(range(N_CORES)),
        trace=bool(getattr(kernel, "_trace", False)),
        tmpdir=getattr(kernel, "_tmpdir", None),
    )
    kernel.last_results = res

    z_sel = np.empty(B)
    v_full = np.zeros(C)
    for k in range(N_CORES):
        s0 = k * ROWS
        du = res.results[k]["du"].astype(np.float64)   # [128, T]
        uo = res.results[k]["uo"].astype(np.float64)   # [1, CW]
        v_full[bases[k] : bases[k] + CW] = uo[0] / 32.0
        for tt in range(T):
            rows = slice(s0 + tt * 128, s0 + (tt + 1) * 128)
            z_sel[rows] = du[:, tt] / 32.0

    # z_sel is in perm-row order; z_nn in original order — matching the
    # reference's own (faithfully replicated) row pairing.
    z_nn = v_full[t]
    loss = np.mean(np.logaddexp(0.0, z_nn - z_sel))
    return np.asarray(loss, dtype=np.float32)
